# revision 16
# baseline (speedup 1.0000x reference)
"""Bahdanau attention kernel for 8 Trainium2 NeuronCores.

Problem shapes (hardcoded): hidden [2, 32, 1024], encoder_outputs [32, 2048, 1024],
Wq/Wk [1024, 1024], bq/bk/wv [1024], bv scalar. Output [32, 1, 1024].

Sharding: data-parallel over batch B=32 -> 4 batches per core, weights replicated.
bv is dropped entirely (softmax is invariant to constant shifts).

Key structure:
- The K-projection (enc @ Wk.T, the dominant 137 GFLOP) runs in bf16 on the PE
  so weight loads overlap the stream (fp32r self-loading matmuls serialize).
- enc is loaded once (fp32), downcast to bf16 on the vector engine, and
  transposed to the required [h, s] layout with the 16-bit XBAR DMA transpose
  (zero PE cost). encT lives as [p, s-tile, h-chunk, 128] so each XBAR write
  lands contiguously; matmuls read it with strided APs.
- The q+bq+bk bias folds into the tanh as a per-partition bias while the
  activation reads the matmul PSUM directly.
- The bf16 natural-layout tiles are kept in SBUF and reused for the final
  attn @ enc einsum on the PE, so enc is read from HBM exactly once.
"""

from contextlib import ExitStack

import numpy as np

import concourse.bacc as bacc
import concourse.bass as bass
import concourse.mybir as mybir
import concourse.tile as tile
from concourse.bass_utils import run_bass_kernel_spmd
from concourse.masks import make_identity

B, S, H = 32, 2048, 1024
NCORES = 8
BPC = B // NCORES  # 4 batches per core
F32 = mybir.dt.float32
BF16 = mybir.dt.bfloat16
HT = H // 128  # 8 chunks of 128 along h or o
ST = S // 128  # 16 s-tiles of 128
SC = S // 512  # 4 s-chunks of 512
Tanh = mybir.ActivationFunctionType.Tanh
Exp = mybir.ActivationFunctionType.Exp
X = mybir.AxisListType.X

ts = bass.ts


def build_program():
    nc = bacc.Bacc("TRN2", target_bir_lowering=False, debug=False)

    hid_d = nc.dram_tensor("hid", [BPC, H], F32, kind="ExternalInput")
    enc_d = nc.dram_tensor("enc", [BPC, S, H], F32, kind="ExternalInput")
    wk_d = nc.dram_tensor("wk", [H, H], F32, kind="ExternalInput")
    wq_d = nc.dram_tensor("wq", [H, H], F32, kind="ExternalInput")
    bq_d = nc.dram_tensor("bq", [1, H], F32, kind="ExternalInput")
    bk_d = nc.dram_tensor("bk", [1, H], F32, kind="ExternalInput")
    wv_d = nc.dram_tensor("wv", [1, H], F32, kind="ExternalInput")
    out_d = nc.dram_tensor("out", [BPC, 1, H], F32, kind="ExternalOutput")

    with tile.TileContext(nc) as tc, ExitStack() as ctx:
        consts = ctx.enter_context(tc.tile_pool(name="consts", bufs=1))
        encnat = ctx.enter_context(tc.tile_pool(name="encnat", bufs=3))
        encbf = ctx.enter_context(tc.tile_pool(name="encbf", bufs=24))
        encT_p = ctx.enter_context(tc.tile_pool(name="encT", bufs=6))
        eT_p = ctx.enter_context(tc.tile_pool(name="eT", bufs=2))
        batch = ctx.enter_context(tc.tile_pool(name="batch", bufs=1))
        tp = ctx.enter_context(tc.tile_pool(name="tp", bufs=2, space="PSUM"))
        kp = ctx.enter_context(tc.tile_pool(name="kp", bufs=4, space="PSUM"))
        vp = ctx.enter_context(tc.tile_pool(name="vp", bufs=2, space="PSUM"))

        setup = tc.tile_pool(name="setup", bufs=2)
        stage = setup.__enter__()

        ident = consts.tile([128, 128], F32, tag="ident")
        make_identity(nc, ident[:])
        ones_bf = consts.tile([1, 128], BF16, tag="ones")
        nc.vector.memset(ones_bf[:], 1.0)

        # ---- Wk -> wkT4[p, o-tile t, h-chunk c, n] bf16 via downcast + XBAR ----
        # wkT4[p, t, c, n] = Wk[128t+n, 128c+p] = Wk^T[h=128c+p, o=128t+n]
        wkT4 = consts.tile([128, HT, HT, 128], BF16, tag="wkT")
        for t in range(HT):
            wnat = stage.tile([128, H], F32, tag="wnat")
            nc.scalar.dma_start(wnat[:], wk_d[ts(t, 128), :])
            wbf = stage.tile([128, H], BF16, tag="wbf")
            nc.vector.tensor_copy(wbf[:], wnat[:])
            nc.sync.dma_start_transpose(wkT4[:, t, :, :], wbf[:])

        # ---- enc staging pipeline, emitted per batch ----
        def stage_enc(b):
            # encTs[j][p, u, c, n] = enc^T[h=128c+p, s=512j+128u+n], bf16
            encTs, ebs = [], []
            for j in range(SC):
                encTj = encT_p.tile([128, 4, HT, 128], BF16, tag="encTj")
                for u in range(4):
                    t = 4 * j + u
                    en = encnat.tile([128, H], F32, tag="encnat")
                    nc.gpsimd.dma_start(en[:], enc_d[b, ts(t, 128), :])
                    eb = encbf.tile([128, H], BF16, tag="encbf")
                    nc.vector.tensor_copy(eb[:], en[:])
                    nc.sync.dma_start_transpose(encTj[:, u, :, :], eb[:])
                    ebs.append(eb)
                encTs.append(encTj)
            return encTs, ebs

        staged = {0: stage_enc(0)}


        # ---- biases: bsum[o(part), o-chunk] = bq + bk ----
        brow = stage.tile([1, H], F32, tag="brow", bufs=1)
        brow2 = stage.tile([1, H], F32, tag="brow2", bufs=1)
        nc.scalar.dma_start(brow[:], bq_d[:])
        nc.scalar.dma_start(brow2[:], bk_d[:])
        nc.vector.tensor_add(brow[:], brow[:], brow2[:])
        bsum = consts.tile([128, HT], F32, tag="bsum")
        for c in range(HT):
            pa = tp.tile([128, 1], F32, tag="tp")
            nc.tensor.transpose(pa[:], brow[0:1, ts(c, 128)], ident[0:1, 0:1])
            nc.vector.tensor_copy(bsum[:, c : c + 1], pa[:])

        # ---- wv -> wvT[o(part), o-chunk] bf16 ----
        wvrow = stage.tile([1, H], F32, tag="wvrow", bufs=1)
        nc.scalar.dma_start(wvrow[:], wv_d[:])
        wvT = consts.tile([128, HT], BF16, tag="wvT")
        for c in range(HT):
            pa = tp.tile([128, 1], F32, tag="tp")
            nc.tensor.transpose(pa[:], wvrow[0:1, ts(c, 128)], ident[0:1, 0:1])
            nc.vector.tensor_copy(wvT[:, c : c + 1], pa[:])

        # ---- hidden slice -> hidT[h(part), h-chunk, b] (fp32) ----
        hid_nat = stage.tile([BPC, H], F32, tag="hidnat", bufs=1)
        nc.scalar.dma_start(hid_nat[:], hid_d[:])
        hidT = consts.tile([128, HT, BPC], F32, tag="hidT")
        for c in range(HT):
            pa = tp.tile([128, BPC], F32, tag="tp")
            nc.tensor.transpose(pa[:], hid_nat[0:BPC, ts(c, 128)], ident[0:BPC, 0:BPC])
            nc.vector.tensor_copy(hidT[:, c, :], pa[:])

        # ---- q^T + bq + bk: qkb[o(part), o-chunk t, b] (fp32 throughout) ----
        qkb = consts.tile([128, HT, BPC], F32, tag="qkb")
        for t in range(HT):
            wnat = stage.tile([128, H], F32, tag="wnat")
            nc.scalar.dma_start(wnat[:], wq_d[ts(t, 128), :])
            pq = kp.tile([128, BPC], F32, tag="kp")
            for c in range(HT):
                blk = tp.tile([128, 128], F32, tag="tp")
                nc.tensor.transpose(blk[:], wnat[:, ts(c, 128)], ident[:])
                blks = stage.tile([128, 128], F32, tag="blks")
                nc.vector.tensor_copy(blks[:], blk[:])
                nc.tensor.matmul(
                    pq[:], blks[:], hidT[:, c, :], start=(c == 0), stop=(c == HT - 1)
                )
            nc.vector.tensor_scalar_add(qkb[:, t, :], pq[:], bsum[:, t : t + 1])

        setup.__exit__(None, None, None)

        # ---- per-batch pieces, emitted in software-pipelined order ----
        def kproj_chunk(b, j, encTs, scores):
            # K^T tiles + fused bias/tanh -> eT_j[o(part), o-chunk i, s(512)]
            eT_j = eT_p.tile([128, HT, 512], BF16, tag="eTj")
            for i in range(HT):
                pk = kp.tile([128, 512], F32, tag="kp")
                for c in range(HT):
                    nc.tensor.matmul(
                        pk[:],
                        wkT4[:, i, c, :],
                        encTs[j][:, :, c, :],
                        start=(c == 0),
                        stop=(c == HT - 1),
                    )
                nc.scalar.activation(
                    eT_j[:, i, :], pk[:], Tanh, bias=qkb[:, i, b : b + 1]
                )

            # scores chunk j = wv . eT_j (contraction over o via PE)
            ps = vp.tile([1, 512], F32, tag="vp")
            for i in range(HT):
                nc.tensor.matmul(
                    ps[:],
                    wvT[:, i : i + 1],
                    eT_j[:, i, :],
                    start=(i == 0),
                    stop=(i == HT - 1),
                )
            nc.vector.tensor_copy(scores[0:1, ts(j, 512)], ps[:])

        def softmax(scores):
            # softmax over s (free dim, single partition); normalized attn bf16
            mx = batch.tile([1, 1], F32, tag="mx")
            nc.vector.reduce_max(mx[:], scores[:], axis=X)
            nmx = batch.tile([1, 1], F32, tag="nmx")
            nc.vector.tensor_scalar_mul(nmx[:], mx[:], -1.0)
            exps = batch.tile([1, S], F32, tag="exps")
            ssum = batch.tile([1, 1], F32, tag="ssum")
            nc.scalar.activation(
                exps[:], scores[:], Exp, bias=nmx[0:1, 0:1], accum_out=ssum[:]
            )
            inv = batch.tile([1, 1], F32, tag="inv")
            nc.vector.reciprocal(inv[:], ssum[:])
            attn = batch.tile([1, S], BF16, tag="attn")
            nc.vector.tensor_scalar_mul(attn[:], exps[:], inv[0:1, 0:1])
            return attn

        def finalize(b, attn, ebs):
            # attn^T columns [s(part), s-tile] bf16
            atT = batch.tile([128, ST], BF16, tag="atT")
            for t in range(ST):
                pa = tp.tile([128, 1], BF16, tag="tp")
                nc.tensor.transpose(pa[:], attn[0:1, ts(t, 128)], ones_bf[0:1, 0:1])
                nc.vector.tensor_copy(atT[:, t : t + 1], pa[:])

            # out[b] = attn @ enc via PE against the kept bf16 natural tiles
            outb = batch.tile([1, H], F32, tag="outb")
            for hc in range(2):
                po = vp.tile([1, 512], F32, tag="vp")
                for t in range(ST):
                    nc.tensor.matmul(
                        po[:],
                        atT[:, t : t + 1],
                        ebs[t][:, ts(hc, 512)],
                        start=(t == 0),
                        stop=(t == ST - 1),
                    )
                nc.vector.tensor_copy(outb[0:1, ts(hc, 512)], po[:])
            nc.gpsimd.dma_start(out_d[b], outb[:])

        # ---- main loop: finalize(b-1) is emitted between chunks of batch b so
        # the PE never stalls on batch b-1's softmax ----
        pending = None
        for b in range(BPC):
            encTs, ebs = staged.pop(b)
            if b + 1 < BPC:
                staged[b + 1] = stage_enc(b + 1)

            scores = batch.tile([1, S], F32, tag="scores")
            for j in range(SC):
                kproj_chunk(b, j, encTs, scores)
                if j == 1 and pending is not None:
                    finalize(*pending)
                    pending = None
            attn = softmax(scores)
            pending = (b, attn, ebs)
        finalize(*pending)

    nc.compile()
    return nc


_CACHED_NC = None


def _get_nc():
    global _CACHED_NC
    if _CACHED_NC is None:
        _CACHED_NC = build_program()
    return _CACHED_NC


def make_in_maps(hidden, encoder_outputs, Wq, bq, Wk, bk, wv):
    hid_last = np.ascontiguousarray(np.asarray(hidden, np.float32)[-1])  # [32, H]
    enc = np.asarray(encoder_outputs, np.float32)
    Wq = np.ascontiguousarray(np.asarray(Wq, np.float32))
    Wk = np.ascontiguousarray(np.asarray(Wk, np.float32))
    bq = np.asarray(bq, np.float32).reshape(1, H)
    bk = np.asarray(bk, np.float32).reshape(1, H)
    wv = np.asarray(wv, np.float32).reshape(1, H)
    in_maps = []
    for c in range(NCORES):
        sl = slice(c * BPC, (c + 1) * BPC)
        in_maps.append(
            {
                "hid": np.ascontiguousarray(hid_last[sl]),
                "enc": np.ascontiguousarray(enc[sl]),
                "wk": Wk,
                "wq": Wq,
                "bq": bq,
                "bk": bk,
                "wv": wv,
            }
        )
    return in_maps


def run(inputs, trace=False):
    """Run on hardware; returns (output [32,1,1024], BassKernelResults)."""
    nc = _get_nc()
    in_maps = make_in_maps(
        inputs["hidden"],
        inputs["encoder_outputs"],
        inputs["Wq"],
        inputs["bq"],
        inputs["Wk"],
        inputs["bk"],
        inputs["wv"],
    )
    res = run_bass_kernel_spmd(nc, in_maps, list(range(NCORES)), trace=trace)
    out = np.concatenate([res.results[c]["out"] for c in range(NCORES)], axis=0)
    return out.reshape(B, 1, H).astype(np.float32), res


def kernel(hidden, encoder_outputs, Wq, bq, Wk, bk, wv, bv):
    out, _ = run(
        {
            "hidden": hidden,
            "encoder_outputs": encoder_outputs,
            "Wq": Wq,
            "bq": bq,
            "Wk": Wk,
            "bk": bk,
            "wv": wv,
        }
    )
    return out


# revision 18
# speedup vs baseline: 1.3533x; 1.3533x over previous
"""Bahdanau attention kernel for 8 Trainium2 NeuronCores.

Problem shapes (hardcoded): hidden [2, 32, 1024], encoder_outputs [32, 2048, 1024],
Wq/Wk [1024, 1024], bq/bk/wv [1024], bv scalar. Output [32, 1, 1024].

Sharding: data-parallel over batch B=32 -> 4 batches per core, weights replicated.
bv is dropped entirely (softmax is invariant to constant shifts).

Key structure:
- The K-projection (enc @ Wk.T, the dominant 137 GFLOP) runs in bf16 on the PE
  so weight loads overlap the stream (fp32r self-loading matmuls serialize).
- enc is loaded once (fp32), downcast to bf16 on the vector engine, and
  transposed to the required [h, s] layout with the 16-bit XBAR DMA transpose
  (zero PE cost). encT lives as [p, s-tile, h-chunk, 128] so each XBAR write
  lands contiguously; matmuls read it with strided APs.
- The q+bq+bk bias folds into the tanh as a per-partition bias while the
  activation reads the matmul PSUM directly.
- The bf16 natural-layout tiles are kept in SBUF and reused for the final
  attn @ enc einsum on the PE, so enc is read from HBM exactly once.
"""

from contextlib import ExitStack

import numpy as np

import concourse.bacc as bacc
import concourse.bass as bass
import concourse.mybir as mybir
import concourse.tile as tile
from concourse.bass_utils import run_bass_kernel_spmd
from concourse.masks import make_identity

B, S, H = 32, 2048, 1024
NCORES = 8
BPC = B // NCORES  # 4 batches per core
F32 = mybir.dt.float32
BF16 = mybir.dt.bfloat16
HT = H // 128  # 8 chunks of 128 along h or o
ST = S // 128  # 16 s-tiles of 128
SC = S // 512  # 4 s-chunks of 512
Tanh = mybir.ActivationFunctionType.Tanh
Exp = mybir.ActivationFunctionType.Exp
X = mybir.AxisListType.X

ts = bass.ts


def build_program():
    nc = bacc.Bacc("TRN2", target_bir_lowering=False, debug=False)

    hid_d = nc.dram_tensor("hid", [BPC, H], F32, kind="ExternalInput")
    enc_d = nc.dram_tensor("enc", [BPC, S, H], F32, kind="ExternalInput")
    wk_d = nc.dram_tensor("wk", [H, H], F32, kind="ExternalInput")
    wq_d = nc.dram_tensor("wq", [H, H], F32, kind="ExternalInput")
    bq_d = nc.dram_tensor("bq", [1, H], F32, kind="ExternalInput")
    bk_d = nc.dram_tensor("bk", [1, H], F32, kind="ExternalInput")
    wv_d = nc.dram_tensor("wv", [1, H], F32, kind="ExternalInput")
    out_d = nc.dram_tensor("out", [BPC, 1, H], F32, kind="ExternalOutput")

    with tile.TileContext(nc) as tc, ExitStack() as ctx:
        consts = ctx.enter_context(tc.tile_pool(name="consts", bufs=1))
        tp = ctx.enter_context(tc.tile_pool(name="tp", bufs=2, space="PSUM"))
        kp = ctx.enter_context(tc.tile_pool(name="kp", bufs=4, space="PSUM"))
        vp = ctx.enter_context(tc.tile_pool(name="vp", bufs=2, space="PSUM"))

        setup = tc.tile_pool(name="setup", bufs=2)
        stage = setup.__enter__()

        ident = consts.tile([128, 128], F32, tag="ident")
        make_identity(nc, ident[:])
        ones_bf = consts.tile([1, 128], BF16, tag="ones")
        nc.vector.memset(ones_bf[:], 1.0)

        # ---- Wk -> wkT4[p, o-tile t, h-chunk c, n] bf16 via downcast + XBAR ----
        # wkT4[p, t, c, n] = Wk[128t+n, 128c+p] = Wk^T[h=128c+p, o=128t+n]
        wkT4 = consts.tile([128, HT, HT, 128], BF16, tag="wkT")
        for t in range(HT):
            wnat = stage.tile([128, H], F32, tag="wnat")
            nc.scalar.dma_start(wnat[:], wk_d[ts(t, 128), :])
            wbf = stage.tile([128, H], BF16, tag="wbf")
            nc.vector.tensor_copy(wbf[:], wnat[:])
            nc.sync.dma_start_transpose(wkT4[:, t, :, :], wbf[:])


        # ---- biases: bsum[o(part), o-chunk] = bq + bk ----
        brow = stage.tile([1, H], F32, tag="brow", bufs=1)
        brow2 = stage.tile([1, H], F32, tag="brow2", bufs=1)
        nc.scalar.dma_start(brow[:], bq_d[:])
        nc.scalar.dma_start(brow2[:], bk_d[:])
        nc.vector.tensor_add(brow[:], brow[:], brow2[:])
        bsum = consts.tile([128, HT], F32, tag="bsum")
        for c in range(HT):
            pa = tp.tile([128, 1], F32, tag="tp")
            nc.tensor.transpose(pa[:], brow[0:1, ts(c, 128)], ident[0:1, 0:1])
            nc.vector.tensor_copy(bsum[:, c : c + 1], pa[:])

        # ---- wv -> wvT[o(part), o-chunk] bf16 ----
        wvrow = stage.tile([1, H], F32, tag="wvrow", bufs=1)
        nc.scalar.dma_start(wvrow[:], wv_d[:])
        wvT = consts.tile([128, HT], BF16, tag="wvT")
        for c in range(HT):
            pa = tp.tile([128, 1], F32, tag="tp")
            nc.tensor.transpose(pa[:], wvrow[0:1, ts(c, 128)], ident[0:1, 0:1])
            nc.vector.tensor_copy(wvT[:, c : c + 1], pa[:])

        # ---- hidden slice -> hidT[h(part), h-chunk, b] (fp32) ----
        hid_nat = stage.tile([BPC, H], F32, tag="hidnat", bufs=1)
        nc.scalar.dma_start(hid_nat[:], hid_d[:])
        hidT = consts.tile([128, HT, BPC], F32, tag="hidT")
        for c in range(HT):
            pa = tp.tile([128, BPC], F32, tag="tp")
            nc.tensor.transpose(pa[:], hid_nat[0:BPC, ts(c, 128)], ident[0:BPC, 0:BPC])
            nc.vector.tensor_copy(hidT[:, c, :], pa[:])

        # ---- q^T + bq + bk: qkb[o(part), o-chunk t, b] (fp32 throughout) ----
        qkb = consts.tile([128, HT, BPC], F32, tag="qkb")
        for t in range(HT):
            wnat = stage.tile([128, H], F32, tag="wnat")
            nc.scalar.dma_start(wnat[:], wq_d[ts(t, 128), :])
            pq = kp.tile([128, BPC], F32, tag="kp")
            for c in range(HT):
                blk = tp.tile([128, 128], F32, tag="tp")
                nc.tensor.transpose(blk[:], wnat[:, ts(c, 128)], ident[:])
                blks = stage.tile([128, 128], F32, tag="blks")
                nc.vector.tensor_copy(blks[:], blk[:])
                nc.tensor.matmul(
                    pq[:], blks[:], hidT[:, c, :], start=(c == 0), stop=(c == HT - 1)
                )
            nc.vector.tensor_scalar_add(qkb[:, t, :], pq[:], bsum[:, t : t + 1])

        setup.__exit__(None, None, None)

        encnat = ctx.enter_context(tc.tile_pool(name="encnat", bufs=2))
        encbf = ctx.enter_context(tc.tile_pool(name="encbf", bufs=6))
        encT_p = ctx.enter_context(tc.tile_pool(name="encT", bufs=5))
        eT_p = ctx.enter_context(tc.tile_pool(name="eT", bufs=2))
        batch = ctx.enter_context(tc.tile_pool(name="batch", bufs=1))

        # ---- enc staging pipeline, emitted per batch ----
        # One 2MB DMA + one cast per 512-row chunk, then 4 XBAR transposes.
        def stage_enc(b):
            # encTs[j][p, u, c, n] = enc^T[h=128c+p, s=512j+128u+n], bf16
            encTs, ebs = [], []
            for j in range(SC):
                en4 = encnat.tile([128, 4, H], F32, tag="encnat")
                nc.gpsimd.dma_start(
                    en4[:], enc_d[b, ts(j, 512), :].rearrange("(u p) h -> p u h", p=128)
                )
                eb4 = encbf.tile([128, 4, H], BF16, tag="encbf")
                nc.vector.tensor_copy(eb4[:], en4[:])
                encTj = encT_p.tile([128, 4, HT, 128], BF16, tag="encTj")
                for u in range(4):
                    nc.sync.dma_start_transpose(encTj[:, u, :, :], eb4[:, u, :])
                encTs.append(encTj)
                ebs.append(eb4)
            return encTs, ebs

        staged = {0: stage_enc(0)}

        # ---- per-batch pieces, emitted in software-pipelined order ----
        def kproj_chunk(b, j, encTs, scores):
            # K^T tiles + fused bias/tanh -> eT_j[o(part), o-chunk i, s(512)]
            eT_j = eT_p.tile([128, HT, 512], BF16, tag="eTj")
            for i in range(HT):
                pk = kp.tile([128, 512], F32, tag="kp")
                for c in range(HT):
                    nc.tensor.matmul(
                        pk[:],
                        wkT4[:, i, c, :],
                        encTs[j][:, :, c, :],
                        start=(c == 0),
                        stop=(c == HT - 1),
                    )
                nc.scalar.activation(
                    eT_j[:, i, :], pk[:], Tanh, bias=qkb[:, i, b : b + 1]
                )

            # scores chunk j = wv . eT_j (contraction over o via PE)
            ps = vp.tile([1, 512], F32, tag="vp")
            for i in range(HT):
                nc.tensor.matmul(
                    ps[:],
                    wvT[:, i : i + 1],
                    eT_j[:, i, :],
                    start=(i == 0),
                    stop=(i == HT - 1),
                )
            nc.vector.tensor_copy(scores[0:1, ts(j, 512)], ps[:])

        def softmax(scores):
            # softmax over s (free dim, single partition); normalized attn bf16
            mx = batch.tile([1, 1], F32, tag="mx")
            nc.vector.reduce_max(mx[:], scores[:], axis=X)
            nmx = batch.tile([1, 1], F32, tag="nmx")
            nc.vector.tensor_scalar_mul(nmx[:], mx[:], -1.0)
            exps = batch.tile([1, S], F32, tag="exps")
            ssum = batch.tile([1, 1], F32, tag="ssum")
            nc.scalar.activation(
                exps[:], scores[:], Exp, bias=nmx[0:1, 0:1], accum_out=ssum[:]
            )
            inv = batch.tile([1, 1], F32, tag="inv")
            nc.vector.reciprocal(inv[:], ssum[:])
            attn = batch.tile([1, S], BF16, tag="attn")
            nc.vector.tensor_scalar_mul(attn[:], exps[:], inv[0:1, 0:1])
            return attn

        def finalize(b, attn, ebs):
            # attn^T columns [s(part), s-tile] bf16
            atT = batch.tile([128, ST], BF16, tag="atT")
            for t in range(ST):
                pa = tp.tile([128, 1], BF16, tag="tp")
                nc.tensor.transpose(pa[:], attn[0:1, ts(t, 128)], ones_bf[0:1, 0:1])
                nc.vector.tensor_copy(atT[:, t : t + 1], pa[:])

            # out[b] = attn @ enc via PE against the kept bf16 natural tiles
            outb = batch.tile([1, H], F32, tag="outb")
            for hc in range(2):
                po = vp.tile([1, 512], F32, tag="vp")
                for t in range(ST):
                    nc.tensor.matmul(
                        po[:],
                        atT[:, t : t + 1],
                        ebs[t // 4][:, t % 4, ts(hc, 512)],
                        start=(t == 0),
                        stop=(t == ST - 1),
                    )
                nc.vector.tensor_copy(outb[0:1, ts(hc, 512)], po[:])
            nc.gpsimd.dma_start(out_d[b], outb[:])

        # ---- main loop: finalize(b-1) is emitted between chunks of batch b so
        # the PE never stalls on batch b-1's softmax ----
        pending = None
        for b in range(BPC):
            encTs, ebs = staged.pop(b)
            if b + 1 < BPC:
                staged[b + 1] = stage_enc(b + 1)

            scores = batch.tile([1, S], F32, tag="scores")
            for j in range(SC):
                kproj_chunk(b, j, encTs, scores)
                if j == 1 and pending is not None:
                    finalize(*pending)
                    pending = None
            attn = softmax(scores)
            pending = (b, attn, ebs)
        finalize(*pending)

    nc.compile()
    return nc


_CACHED_NC = None


def _get_nc():
    global _CACHED_NC
    if _CACHED_NC is None:
        _CACHED_NC = build_program()
    return _CACHED_NC


def make_in_maps(hidden, encoder_outputs, Wq, bq, Wk, bk, wv):
    hid_last = np.ascontiguousarray(np.asarray(hidden, np.float32)[-1])  # [32, H]
    enc = np.asarray(encoder_outputs, np.float32)
    Wq = np.ascontiguousarray(np.asarray(Wq, np.float32))
    Wk = np.ascontiguousarray(np.asarray(Wk, np.float32))
    bq = np.asarray(bq, np.float32).reshape(1, H)
    bk = np.asarray(bk, np.float32).reshape(1, H)
    wv = np.asarray(wv, np.float32).reshape(1, H)
    in_maps = []
    for c in range(NCORES):
        sl = slice(c * BPC, (c + 1) * BPC)
        in_maps.append(
            {
                "hid": np.ascontiguousarray(hid_last[sl]),
                "enc": np.ascontiguousarray(enc[sl]),
                "wk": Wk,
                "wq": Wq,
                "bq": bq,
                "bk": bk,
                "wv": wv,
            }
        )
    return in_maps


def run(inputs, trace=False):
    """Run on hardware; returns (output [32,1,1024], BassKernelResults)."""
    nc = _get_nc()
    in_maps = make_in_maps(
        inputs["hidden"],
        inputs["encoder_outputs"],
        inputs["Wq"],
        inputs["bq"],
        inputs["Wk"],
        inputs["bk"],
        inputs["wv"],
    )
    res = run_bass_kernel_spmd(nc, in_maps, list(range(NCORES)), trace=trace)
    out = np.concatenate([res.results[c]["out"] for c in range(NCORES)], axis=0)
    return out.reshape(B, 1, H).astype(np.float32), res


def kernel(hidden, encoder_outputs, Wq, bq, Wk, bk, wv, bv):
    out, _ = run(
        {
            "hidden": hidden,
            "encoder_outputs": encoder_outputs,
            "Wq": Wq,
            "bq": bq,
            "Wk": Wk,
            "bk": bk,
            "wv": wv,
        }
    )
    return out
